# revision 24
# baseline (speedup 1.0000x reference)
"""NonLocalBlock (non-local attention) Trainium2 kernel.

Problem: x[4, 256, 64, 64] f32; 1x1-conv projections theta/phi/g (mid=128),
attention over n = h*w = 4096 positions per batch element, output projection
and residual add.

Sharding: data-parallel over batch (4) x query-halves (2) = 8 cores.
Each core gets the full key/value side (xf = x[b] as [256, 4096]) and a
2048-query slice; it computes out[b][:, i_range] = x + w_out @ y.

Per-core algorithm (all layouts chosen to avoid on-chip transposes):
  - theta[m, i] = w_theta @ xq, phi[m, j] = w_phi @ xf  (native fp32 matmuls)
    then split into fp16 hi/lo pairs so the big scores matmul runs as three
    full-rate 16-bit passes (hi*hi + hi*lo + lo*hi); the dropped lo*lo term
    is ~2^-22 relative, i.e. fp32-grade scores.
  - g^T[j, m] computed directly transposed: lhsT = xf block, rhs = w_g^T.
  - scores_T[j, i] = phi^T theta per (j-block 128, i-tile 512); two j-blocks
    share one 2-bank PSUM tile so exp / denominator-accumulate run as one
    [128, 1024] instruction each (halves ACT/DVE per-instruction overhead).
    exp on ACT f32->bf16 with a constant -30 bias instead of a row-max
    subtraction (softmax-invariant; scores reach ~+-80 here, ACT Exp is
    accurate to ~1e-5 over that range, and the shift keeps exp and the
    denominator sums far from f32/bf16 overflow).
  - denominator: DVE accumulates exp tiles into acc (f32), then ones-vector
    matmuls reduce over partitions; reciprocal; K=1 ones matmul broadcasts
    1/denom to all partitions.
  - y[m, i] accumulated over 32 j-blocks in PSUM: lhsT = g^T fp16 block,
    rhs = exp bf16 tile (mixed 16-bit matmul, exact); then scaled by the
    broadcast 1/denom on DVE.
  - out = w_out @ y (fp32) + xq residual, DMA to DRAM.
"""

import numpy as np

import concourse.bacc as bacc
import concourse.bass as bass
import concourse.mybir as mybir
import concourse.tile as tile
from concourse.bass import ts
from concourse.bass_utils import run_bass_kernel_spmd

F32 = mybir.dt.float32
F16 = mybir.dt.float16
BF16 = mybir.dt.bfloat16
AF = mybir.ActivationFunctionType
OP = mybir.AluOpType

P = 128
C = 256  # in channels
MID = 128  # projection channels
N = 4096  # keys per batch element (h*w)
I = 2048  # queries per core
FD = 512  # matmul free-dim tile
FD2 = 1024  # paired tile width
NB_J = N // P  # 32 j-blocks
NT_I = I // FD  # 4 i-tiles


def _build_program() -> bass.Bass:
    # Bacc (not plain Bass): its compile() pass pipeline legalizes semaphore
    # waits (generate_event_semaphores) for this compiler's 1-wait-per-
    # instruction limit.
    nc = bacc.Bacc("TRN2", target_bir_lowering=False)
    xf = nc.dram_tensor("xf", [C, N], F32, kind="ExternalInput")
    xq = nc.dram_tensor("xq", [C, I], F32, kind="ExternalInput")
    # wcat = [w_theta.T | w_phi.T | w_g.T] concatenated on axis 1 so all
    # projection weights arrive in one DMA per 128-channel block.
    wcat = nc.dram_tensor("wcat", [C, 3 * MID], F32, kind="ExternalInput")
    wo = nc.dram_tensor("wo_t", [MID, C], F32, kind="ExternalInput")
    out = nc.dram_tensor("out", [C, I], F32, kind="ExternalOutput")

    with tile.TileContext(nc) as tc:
        with (
            tc.tile_pool(name="singles", bufs=1) as singles,
            tc.tile_pool(name="work", bufs=4) as work,
            tc.tile_pool(name="expp", bufs=12) as expp,
            tc.tile_pool(name="ps", bufs=2, space="PSUM") as ps_pool,
            tc.tile_pool(name="psy", bufs=2, space="PSUM") as psy_pool,
            tc.tile_pool(name="pstail", bufs=2, space="PSUM") as tail_pool,
        ):
            # ---- stage 0: loads -------------------------------------------
            # Loads spread over three DMA queues, ordered so each
            # projection's inputs land just before use:
            #   SP (sync):     wcat[cb=0], xf[cb=0] tiles, wo
            #   Pool (gpsimd): wcat[cb=1], xf[cb=1] tiles
            #   ACT (scalar):  xq tiles, issued from inside the stage-1 loop
            wcat_sb = [
                singles.tile([P, 3 * MID], F32, tag=f"wcat{cb}", name=f"wcat_sb{cb}")
                for cb in range(2)
            ]
            nc.sync.dma_start(wcat_sb[0][:], wcat[ts(0, P), :])
            nc.gpsimd.dma_start(wcat_sb[1][:], wcat[ts(1, P), :])
            wth_sb = [wcat_sb[cb][:, 0:MID] for cb in range(2)]
            wph_sb = [wcat_sb[cb][:, MID : 2 * MID] for cb in range(2)]
            wg_sb = [wcat_sb[cb][:, 2 * MID : 3 * MID] for cb in range(2)]

            # xf tiles per channel-block on SP (cb=0) and the gpsimd SWDGE
            # queue (cb=1). xq tiles go on the ACT queue but are issued
            # inside the stage-1 loop (one pair ahead of use) so the issue
            # cost interleaves with ACT compute instead of blocking it.
            xq_sb = [
                singles.tile([P, I], F32, tag=f"xq{cb}", name=f"xq_sb{cb}")
                for cb in range(2)
            ]
            for cb in range(2):
                nc.scalar.dma_start(xq_sb[cb][:, ts(0, FD)], xq[ts(cb, P), ts(0, FD)])
            xf_sb = [
                singles.tile([P, N], F32, tag=f"xf{cb}", name=f"xf_sb{cb}")
                for cb in range(2)
            ]
            engines = [nc.sync, nc.gpsimd]
            for k in range(N // FD):
                for cb in range(2):
                    engines[cb].dma_start(
                        xf_sb[cb][:, ts(k, FD)], xf[ts(cb, P), ts(k, FD)]
                    )
            wo_sb = singles.tile([P, C], F32, tag="wo")
            nc.sync.dma_start(wo_sb[:], wo[:, :])

            ones_col = singles.tile([P, 1], F32, tag="ones_col")
            nc.vector.memset(ones_col[:], 1.0)
            neg30 = singles.tile([P, 1], F32, tag="neg30")
            nc.vector.memset(neg30[:], -30.0)
            ones_row = singles.tile([1, P], F32, tag="ones_row")
            nc.vector.memset(ones_row[:], 1.0)

            # ---- stage 1: projections -------------------------------------
            # Emission interleaved by arrival order of the input tiles so the
            # PE never waits long: per key-tile jt do phi(jt), the fp16 casts,
            # and the four g^T blocks it enables; theta(it=jt) for jt < 4.
            #
            # theta/phi are split into fp16 hi/lo pairs; g^T is computed from
            # fp16 casts of xf / w_g (full-rate matmuls; ~3e-4 elementwise
            # error in g, negligible downstream). The y matmul later takes
            # fp16 stationary x bf16 moving, which is exact on the PE.
            th_hi = singles.tile([P, I], F16, tag="th_hi")
            th_lo = singles.tile([P, I], F16, tag="th_lo")
            ph_hi = singles.tile([P, N], F16, tag="ph_hi")
            ph_lo = singles.tile([P, N], F16, tag="ph_lo")
            gt = singles.tile([P, NB_J, MID], F16, tag="gt")
            xf_f16 = [
                singles.tile([P, N], F16, tag=f"xf16_{cb}", name=f"xf_f16_{cb}")
                for cb in range(2)
            ]
            wg_f16 = [
                singles.tile([P, MID], F16, tag=f"wg16_{cb}", name=f"wg_f16_{cb}")
                for cb in range(2)
            ]
            for cb in range(2):
                nc.scalar.copy(wg_f16[cb][:], wg_sb[cb][:])

            for jt in range(N // FD):
                sl = ts(jt, FD)
                # phi(jt)
                psp = ps_pool.tile([P, FD2], F32, tag="pss")
                nc.tensor.matmul(
                    psp[:, :FD], wph_sb[0][:], xf_sb[0][:, sl],
                    start=True, stop=False,
                )
                nc.tensor.matmul(
                    psp[:, :FD], wph_sb[1][:], xf_sb[1][:, sl],
                    start=False, stop=True,
                )
                nc.scalar.copy(ph_hi[:, sl], psp[:, :FD])
                nc.vector.tensor_tensor(
                    ph_lo[:, sl], psp[:, :FD], ph_hi[:, sl], op=OP.subtract
                )

                # theta(it=jt) for the first four tiles
                if jt < NT_I:
                    if jt + 1 < NT_I:
                        nsl = ts(jt + 1, FD)
                        for cb in range(2):
                            nc.scalar.dma_start(
                                xq_sb[cb][:, nsl], xq[ts(cb, P), nsl]
                            )
                    pst = ps_pool.tile([P, FD2], F32, tag="pss")
                    nc.tensor.matmul(
                        pst[:, :FD], wth_sb[0][:], xq_sb[0][:, sl],
                        start=True, stop=False,
                    )
                    nc.tensor.matmul(
                        pst[:, :FD], wth_sb[1][:], xq_sb[1][:, sl],
                        start=False, stop=True,
                    )
                    nc.scalar.copy(th_hi[:, sl], pst[:, :FD])
                    nc.vector.tensor_tensor(
                        th_lo[:, sl], pst[:, :FD], th_hi[:, sl], op=OP.subtract
                    )

                # fp16 casts of this xf tile, then the 4 g^T blocks it covers
                nc.scalar.copy(xf_f16[0][:, sl], xf_sb[0][:, sl])
                nc.gpsimd.tensor_copy(xf_f16[1][:, sl], xf_sb[1][:, sl])
                for jb in range(4 * jt, 4 * jt + 4):
                    psg = tail_pool.tile([P, FD], F32, tag="tail")
                    nc.tensor.matmul(
                        psg[:, :MID], xf_f16[0][:, ts(jb, P)], wg_f16[0][:],
                        start=True, stop=False,
                    )
                    nc.tensor.matmul(
                        psg[:, :MID], xf_f16[1][:, ts(jb, P)], wg_f16[1][:],
                        start=False, stop=True,
                    )
                    nc.vector.tensor_copy(gt[:, jb, :], psg[:, :MID])

            # ---- stage 2: attention ---------------------------------------
            for it in range(NT_I):
                th_sl = ts(it, FD)
                psy = psy_pool.tile([P, FD], F32, tag="psy")
                acc = work.tile([P, FD2], F32, tag="acc")
                nc.vector.memset(acc[:], 0.0)
                for pr in range(NB_J // 2):
                    pss = ps_pool.tile([P, FD2], F32, tag="pss")
                    for h in range(2):
                        jb = 2 * pr + h
                        sl = ts(h, FD)
                        nc.tensor.matmul(
                            pss[:, sl], ph_hi[:, ts(jb, P)], th_hi[:, th_sl],
                            start=True, stop=False,
                        )
                        nc.tensor.matmul(
                            pss[:, sl], ph_hi[:, ts(jb, P)], th_lo[:, th_sl],
                            start=False, stop=False,
                        )
                        nc.tensor.matmul(
                            pss[:, sl], ph_lo[:, ts(jb, P)], th_hi[:, th_sl],
                            start=False, stop=True,
                        )
                    e = expp.tile([P, FD2], BF16, tag="e")
                    # constant -30 bias: softmax-invariant, keeps exp and the
                    # denominator far from f32/bf16 overflow (scores reach
                    # ~±80 on N(0,1)-scale inputs; without a shift the
                    # denominator sum comes within ~10x of f32 max)
                    nc.scalar.activation(e[:], pss[:], func=AF.Exp, bias=neg30[:])
                    # denominator partials, both halves at once (f32 += bf16)
                    nc.vector.tensor_tensor(acc[:], acc[:], e[:], op=OP.add)
                    # y accumulation: psy[m, i] += gt[jb]^T @ e[jb half]
                    for h in range(2):
                        jb = 2 * pr + h
                        nc.tensor.matmul(
                            psy[:], gt[:, jb, :], e[:, ts(h, FD)],
                            start=(jb == 0), stop=(jb == NB_J - 1),
                        )

                # denominator: reduce acc over partitions (both halves
                # into one accumulation group), recip, broadcast
                psd = tail_pool.tile([P, FD], F32, tag="tail")
                nc.tensor.matmul(
                    psd[:1, :], ones_col[:], acc[:, :FD], start=True, stop=False
                )
                nc.tensor.matmul(
                    psd[:1, :], ones_col[:], acc[:, FD:], start=False, stop=True
                )
                rden = work.tile([1, FD], F32, tag="rden")
                nc.vector.reciprocal(rden[:], psd[:1, :])
                psb = tail_pool.tile([P, FD], F32, tag="tail")
                nc.tensor.matmul(
                    psb[:], ones_row[:], rden[:], start=True, stop=True
                )
                rb_sb = work.tile([P, FD], F32, tag="rb")
                nc.scalar.copy(rb_sb[:], psb[:])

                # normalize y while copying PSUM -> SBUF
                y_sb = work.tile([P, FD], F32, tag="y")
                nc.vector.tensor_tensor(y_sb[:], psy[:], rb_sb[:], op=OP.mult)

                # out = w_out @ y + xq (residual), per 128-channel block
                for cb in range(2):
                    pso = tail_pool.tile([P, FD], F32, tag="tail")
                    nc.tensor.matmul(
                        pso[:], wo_sb[:, ts(cb, P)], y_sb[:],
                        start=True, stop=True,
                    )
                    o_sb = work.tile([P, FD], F32, tag="o")
                    nc.vector.tensor_tensor(
                        o_sb[:], pso[:], xq_sb[cb][:, th_sl], op=OP.add
                    )
                    nc.sync.dma_start(
                        out[ts(cb, P), it * FD : it * FD + 256], o_sb[:, :256]
                    )
                    nc.gpsimd.dma_start(
                        out[ts(cb, P), it * FD + 256 : (it + 1) * FD],
                        o_sb[:, 256:],
                    )

    nc.compile()
    return nc


_CACHED_NC = None


def _get_program():
    global _CACHED_NC
    if _CACHED_NC is None:
        _CACHED_NC = _build_program()
    return _CACHED_NC


def make_in_maps(x, w_theta, w_phi, w_g, w_out):
    b, c, h, w = x.shape
    n = h * w
    assert (b, c, n) == (4, C, N), (b, c, n)
    xf = np.ascontiguousarray(x.reshape(b, c, n), dtype=np.float32)
    wcat = np.ascontiguousarray(
        np.concatenate(
            [
                np.asarray(w_theta, dtype=np.float32).T,
                np.asarray(w_phi, dtype=np.float32).T,
                np.asarray(w_g, dtype=np.float32).T,
            ],
            axis=1,
        )
    )
    wo_t = np.ascontiguousarray(np.asarray(w_out, dtype=np.float32).T)

    in_maps = []
    for core in range(8):
        bb, q = divmod(core, 2)
        in_maps.append(
            {
                "xf": xf[bb],
                "xq": np.ascontiguousarray(xf[bb][:, q * I : (q + 1) * I]),
                "wcat": wcat,
                "wo_t": wo_t,
            }
        )
    return in_maps


def kernel(x, w_theta, w_phi, w_g, w_out):
    b = x.shape[0]
    c = x.shape[1]
    n = x.shape[2] * x.shape[3]
    in_maps = make_in_maps(x, w_theta, w_phi, w_g, w_out)

    nc = _get_program()
    res = run_bass_kernel_spmd(nc, in_maps, core_ids=list(range(8)))

    out_full = np.empty((b, c, n), dtype=np.float32)
    for core in range(8):
        bb, q = divmod(core, 2)
        out_full[bb][:, q * I : (q + 1) * I] = res.results[core]["out"]
    return out_full.reshape(x.shape)


# revision 27
# speedup vs baseline: 1.0096x; 1.0096x over previous
"""NonLocalBlock (non-local attention) Trainium2 kernel.

Problem: x[4, 256, 64, 64] f32; 1x1-conv projections theta/phi/g (mid=128),
attention over n = h*w = 4096 positions per batch element, output projection
and residual add.

Sharding: data-parallel over batch (4) x query-halves (2) = 8 cores.
Each core gets the full key/value side (xf = x[b] as [256, 4096]) and a
2048-query slice; it computes out[b][:, i_range] = x + w_out @ y.

Per-core algorithm (all layouts chosen to avoid on-chip transposes):
  - theta[m, i] = w_theta @ xq, phi[m, j] = w_phi @ xf  (native fp32 matmuls)
    then split into fp16 hi/lo pairs so the big scores matmul runs as three
    full-rate 16-bit passes (hi*hi + hi*lo + lo*hi); the dropped lo*lo term
    is ~2^-22 relative, i.e. fp32-grade scores.
  - g^T[j, m] computed directly transposed: lhsT = xf block, rhs = w_g^T.
  - scores_T[j, i] = phi^T theta per (j-block 128, i-tile 512); two j-blocks
    share one 2-bank PSUM tile so exp / denominator-accumulate run as one
    [128, 1024] instruction each (halves ACT/DVE per-instruction overhead).
    exp on ACT f32->bf16 with a constant -30 bias instead of a row-max
    subtraction (softmax-invariant; scores reach ~+-80 here, ACT Exp is
    accurate to ~1e-5 over that range, and the shift keeps exp and the
    denominator sums far from f32/bf16 overflow).
  - denominator: DVE accumulates exp tiles into acc (f32), then ones-vector
    matmuls reduce over partitions; reciprocal; K=1 ones matmul broadcasts
    1/denom to all partitions.
  - y[m, i] accumulated over 32 j-blocks in PSUM: lhsT = g^T fp16 block,
    rhs = exp bf16 tile (mixed 16-bit matmul, exact); then scaled by the
    broadcast 1/denom on DVE.
  - out = w_out @ y (fp32) + xq residual, DMA to DRAM.
"""

import numpy as np

import concourse.bacc as bacc
import concourse.bass as bass
import concourse.mybir as mybir
import concourse.tile as tile
from concourse.bass import ts
from concourse.bass_utils import run_bass_kernel_spmd

F32 = mybir.dt.float32
F16 = mybir.dt.float16
BF16 = mybir.dt.bfloat16
AF = mybir.ActivationFunctionType
OP = mybir.AluOpType

P = 128
C = 256  # in channels
MID = 128  # projection channels
N = 4096  # keys per batch element (h*w)
I = 2048  # queries per core
FD = 512  # matmul free-dim tile
FD2 = 1024  # paired tile width
NB_J = N // P  # 32 j-blocks
NT_I = I // FD  # 4 i-tiles


def _build_program() -> bass.Bass:
    # Bacc (not plain Bass): its compile() pass pipeline legalizes semaphore
    # waits (generate_event_semaphores) for this compiler's 1-wait-per-
    # instruction limit.
    nc = bacc.Bacc("TRN2", target_bir_lowering=False)
    xf = nc.dram_tensor("xf", [C, N], F32, kind="ExternalInput")
    xq = nc.dram_tensor("xq", [C, I], F32, kind="ExternalInput")
    # wcat = [w_theta.T | w_phi.T | w_g.T] concatenated on axis 1 so all
    # projection weights arrive in one DMA per 128-channel block.
    wcat = nc.dram_tensor("wcat", [C, 3 * MID], F32, kind="ExternalInput")
    wo = nc.dram_tensor("wo_t", [MID, C], F32, kind="ExternalInput")
    out = nc.dram_tensor("out", [C, I], F32, kind="ExternalOutput")

    with tile.TileContext(nc) as tc:
        with (
            tc.tile_pool(name="singles", bufs=1) as singles,
            tc.tile_pool(name="work", bufs=4) as work,
            tc.tile_pool(name="expp", bufs=12) as expp,
            tc.tile_pool(name="ps", bufs=2, space="PSUM") as ps_pool,
            tc.tile_pool(name="psy", bufs=2, space="PSUM") as psy_pool,
            tc.tile_pool(name="pstail", bufs=2, space="PSUM") as tail_pool,
        ):
            # ---- stage 0: loads -------------------------------------------
            # Loads spread over three DMA queues, ordered so each
            # projection's inputs land just before use:
            #   SP (sync):     wcat[cb=0], xf[cb=0] tiles, wo
            #   Pool (gpsimd): wcat[cb=1], xf[cb=1] tiles
            #   ACT (scalar):  xq tiles, issued from inside the stage-1 loop
            wcat_sb = [
                singles.tile([P, 3 * MID], F32, tag=f"wcat{cb}", name=f"wcat_sb{cb}")
                for cb in range(2)
            ]
            # wcat0 on the ACT queue (small, and it frees SP to start xf0[0]
            # immediately — the first phi matmul's critical input)
            nc.scalar.dma_start(wcat_sb[0][:], wcat[ts(0, P), :])
            nc.gpsimd.dma_start(wcat_sb[1][:], wcat[ts(1, P), :])
            wth_sb = [wcat_sb[cb][:, 0:MID] for cb in range(2)]
            wph_sb = [wcat_sb[cb][:, MID : 2 * MID] for cb in range(2)]
            wg_sb = [wcat_sb[cb][:, 2 * MID : 3 * MID] for cb in range(2)]

            # xf tiles per channel-block on SP (cb=0) and the gpsimd SWDGE
            # queue (cb=1). xq tiles go on the ACT queue but are issued
            # inside the stage-1 loop (one pair ahead of use) so the issue
            # cost interleaves with ACT compute instead of blocking it.
            xq_sb = [
                singles.tile([P, I], F32, tag=f"xq{cb}", name=f"xq_sb{cb}")
                for cb in range(2)
            ]
            for cb in range(2):
                nc.scalar.dma_start(xq_sb[cb][:, ts(0, FD)], xq[ts(cb, P), ts(0, FD)])
            xf_sb = [
                singles.tile([P, N], F32, tag=f"xf{cb}", name=f"xf_sb{cb}")
                for cb in range(2)
            ]
            engines = [nc.sync, nc.gpsimd]
            for k in range(N // FD):
                for cb in range(2):
                    engines[cb].dma_start(
                        xf_sb[cb][:, ts(k, FD)], xf[ts(cb, P), ts(k, FD)]
                    )
            wo_sb = singles.tile([P, C], F32, tag="wo")
            nc.sync.dma_start(wo_sb[:], wo[:, :])

            ones_col = singles.tile([P, 1], F32, tag="ones_col")
            nc.vector.memset(ones_col[:], 1.0)
            neg30 = singles.tile([P, 1], F32, tag="neg30")
            nc.vector.memset(neg30[:], -30.0)
            ones_row = singles.tile([1, P], F32, tag="ones_row")
            nc.vector.memset(ones_row[:], 1.0)

            # ---- stage 1: projections -------------------------------------
            # Emission interleaved by arrival order of the input tiles so the
            # PE never waits long: per key-tile jt do phi(jt), the fp16 casts,
            # and the four g^T blocks it enables; theta(it=jt) for jt < 4.
            #
            # theta/phi are split into fp16 hi/lo pairs; g^T is computed from
            # fp16 casts of xf / w_g (full-rate matmuls; ~3e-4 elementwise
            # error in g, negligible downstream). The y matmul later takes
            # fp16 stationary x bf16 moving, which is exact on the PE.
            th_hi = singles.tile([P, I], F16, tag="th_hi")
            th_lo = singles.tile([P, I], F16, tag="th_lo")
            ph_hi = singles.tile([P, N], F16, tag="ph_hi")
            ph_lo = singles.tile([P, N], F16, tag="ph_lo")
            gt = singles.tile([P, NB_J, MID], F16, tag="gt")
            xf_f16 = [
                singles.tile([P, N], F16, tag=f"xf16_{cb}", name=f"xf_f16_{cb}")
                for cb in range(2)
            ]
            wg_f16 = [
                singles.tile([P, MID], F16, tag=f"wg16_{cb}", name=f"wg_f16_{cb}")
                for cb in range(2)
            ]
            for cb in range(2):
                nc.scalar.copy(wg_f16[cb][:], wg_sb[cb][:])

            for jt in range(N // FD):
                sl = ts(jt, FD)
                # phi(jt)
                psp = ps_pool.tile([P, FD2], F32, tag="pss")
                nc.tensor.matmul(
                    psp[:, :FD], wph_sb[0][:], xf_sb[0][:, sl],
                    start=True, stop=False,
                )
                nc.tensor.matmul(
                    psp[:, :FD], wph_sb[1][:], xf_sb[1][:, sl],
                    start=False, stop=True,
                )
                nc.scalar.copy(ph_hi[:, sl], psp[:, :FD])
                nc.vector.tensor_tensor(
                    ph_lo[:, sl], psp[:, :FD], ph_hi[:, sl], op=OP.subtract
                )

                # theta(it=jt) for the first four tiles
                if jt < NT_I:
                    if jt + 1 < NT_I:
                        nsl = ts(jt + 1, FD)
                        for cb in range(2):
                            nc.scalar.dma_start(
                                xq_sb[cb][:, nsl], xq[ts(cb, P), nsl]
                            )
                    pst = ps_pool.tile([P, FD2], F32, tag="pss")
                    nc.tensor.matmul(
                        pst[:, :FD], wth_sb[0][:], xq_sb[0][:, sl],
                        start=True, stop=False,
                    )
                    nc.tensor.matmul(
                        pst[:, :FD], wth_sb[1][:], xq_sb[1][:, sl],
                        start=False, stop=True,
                    )
                    nc.scalar.copy(th_hi[:, sl], pst[:, :FD])
                    nc.vector.tensor_tensor(
                        th_lo[:, sl], pst[:, :FD], th_hi[:, sl], op=OP.subtract
                    )

                # fp16 casts of this xf tile, then the 4 g^T blocks it covers
                nc.scalar.copy(xf_f16[0][:, sl], xf_sb[0][:, sl])
                nc.gpsimd.tensor_copy(xf_f16[1][:, sl], xf_sb[1][:, sl])
                for jb in range(4 * jt, 4 * jt + 4):
                    psg = tail_pool.tile([P, FD], F32, tag="tail")
                    nc.tensor.matmul(
                        psg[:, :MID], xf_f16[0][:, ts(jb, P)], wg_f16[0][:],
                        start=True, stop=False,
                    )
                    nc.tensor.matmul(
                        psg[:, :MID], xf_f16[1][:, ts(jb, P)], wg_f16[1][:],
                        start=False, stop=True,
                    )
                    nc.vector.tensor_copy(gt[:, jb, :], psg[:, :MID])

            # ---- stage 2: attention ---------------------------------------
            for it in range(NT_I):
                th_sl = ts(it, FD)
                last_it = it == NT_I - 1
                # For the last i-tile (whose tail nothing overlaps), the final
                # j-pair is processed as two single-block units feeding a
                # separate small accumulator, so the main denominator matmuls
                # issue early and the end-of-kernel critical chain is short.
                n_pairs = NB_J // 2 - 1 if last_it else NB_J // 2
                psy = psy_pool.tile([P, FD], F32, tag="psy")
                acc = work.tile([P, FD2], F32, tag="acc")
                nc.vector.memset(acc[:], 0.0)
                for pr in range(n_pairs):
                    pss = ps_pool.tile([P, FD2], F32, tag="pss")
                    for h in range(2):
                        jb = 2 * pr + h
                        sl = ts(h, FD)
                        nc.tensor.matmul(
                            pss[:, sl], ph_hi[:, ts(jb, P)], th_hi[:, th_sl],
                            start=True, stop=False,
                        )
                        nc.tensor.matmul(
                            pss[:, sl], ph_hi[:, ts(jb, P)], th_lo[:, th_sl],
                            start=False, stop=False,
                        )
                        nc.tensor.matmul(
                            pss[:, sl], ph_lo[:, ts(jb, P)], th_hi[:, th_sl],
                            start=False, stop=True,
                        )
                    e = expp.tile([P, FD2], BF16, tag="e")
                    # constant -30 bias: softmax-invariant, keeps exp and the
                    # denominator far from f32/bf16 overflow (scores reach
                    # ~±80 on N(0,1)-scale inputs; without a shift the
                    # denominator sum comes within ~10x of f32 max)
                    nc.scalar.activation(e[:], pss[:], func=AF.Exp, bias=neg30[:])
                    # denominator partials, both halves at once (f32 += bf16)
                    nc.vector.tensor_tensor(acc[:], acc[:], e[:], op=OP.add)
                    # y accumulation: psy[m, i] += gt[jb]^T @ e[jb half]
                    for h in range(2):
                        jb = 2 * pr + h
                        nc.tensor.matmul(
                            psy[:], gt[:, jb, :], e[:, ts(h, FD)],
                            start=(jb == 0), stop=(jb == NB_J - 1),
                        )

                # denominator: reduce acc over partitions, recip, broadcast
                psd = tail_pool.tile([P, FD], F32, tag="tail")
                if last_it:
                    acc_last = work.tile([P, FD], F32, tag="accl")
                    nc.vector.memset(acc_last[:], 0.0)
                    for jb in range(2 * n_pairs, NB_J):
                        pss1 = ps_pool.tile([P, FD2], F32, tag="pss")
                        nc.tensor.matmul(
                            pss1[:, :FD], ph_hi[:, ts(jb, P)], th_hi[:, th_sl],
                            start=True, stop=False,
                        )
                        nc.tensor.matmul(
                            pss1[:, :FD], ph_hi[:, ts(jb, P)], th_lo[:, th_sl],
                            start=False, stop=False,
                        )
                        nc.tensor.matmul(
                            pss1[:, :FD], ph_lo[:, ts(jb, P)], th_hi[:, th_sl],
                            start=False, stop=True,
                        )
                        if jb == 2 * n_pairs:
                            # main-acc reduction issues here: acc has long
                            # been final, and the PE still has the singles'
                            # score/y matmuls to chew on afterwards
                            nc.tensor.matmul(
                                psd[:1, :], ones_col[:], acc[:, :FD],
                                start=True, stop=False,
                            )
                            nc.tensor.matmul(
                                psd[:1, :], ones_col[:], acc[:, FD:],
                                start=False, stop=False,
                            )
                        e1 = work.tile([P, FD], BF16, tag="e1")
                        nc.scalar.activation(
                            e1[:], pss1[:, :FD], func=AF.Exp, bias=neg30[:]
                        )
                        nc.vector.tensor_tensor(
                            acc_last[:], acc_last[:], e1[:], op=OP.add
                        )
                        nc.tensor.matmul(
                            psy[:], gt[:, jb, :], e1[:],
                            start=False, stop=(jb == NB_J - 1),
                        )
                    nc.tensor.matmul(
                        psd[:1, :], ones_col[:], acc_last[:],
                        start=False, stop=True,
                    )
                else:
                    nc.tensor.matmul(
                        psd[:1, :], ones_col[:], acc[:, :FD],
                        start=True, stop=False,
                    )
                    nc.tensor.matmul(
                        psd[:1, :], ones_col[:], acc[:, FD:],
                        start=False, stop=True,
                    )
                rden = work.tile([1, FD], F32, tag="rden")
                nc.vector.reciprocal(rden[:], psd[:1, :])
                psb = tail_pool.tile([P, FD], F32, tag="tail")
                nc.tensor.matmul(
                    psb[:], ones_row[:], rden[:], start=True, stop=True
                )
                rb_sb = work.tile([P, FD], F32, tag="rb")
                nc.scalar.copy(rb_sb[:], psb[:])

                # normalize y while copying PSUM -> SBUF
                y_sb = work.tile([P, FD], F32, tag="y")
                nc.vector.tensor_tensor(y_sb[:], psy[:], rb_sb[:], op=OP.mult)

                # out = w_out @ y + xq (residual), per 128-channel block
                for cb in range(2):
                    pso = tail_pool.tile([P, FD], F32, tag="tail")
                    nc.tensor.matmul(
                        pso[:], wo_sb[:, ts(cb, P)], y_sb[:],
                        start=True, stop=True,
                    )
                    o_sb = work.tile([P, FD], F32, tag="o")
                    nc.vector.tensor_tensor(
                        o_sb[:], pso[:], xq_sb[cb][:, th_sl], op=OP.add
                    )
                    nc.sync.dma_start(
                        out[ts(cb, P), it * FD : it * FD + 256], o_sb[:, :256]
                    )
                    nc.gpsimd.dma_start(
                        out[ts(cb, P), it * FD + 256 : (it + 1) * FD],
                        o_sb[:, 256:],
                    )

    nc.compile()
    return nc


_CACHED_NC = None


def _get_program():
    global _CACHED_NC
    if _CACHED_NC is None:
        _CACHED_NC = _build_program()
    return _CACHED_NC


def make_in_maps(x, w_theta, w_phi, w_g, w_out):
    b, c, h, w = x.shape
    n = h * w
    assert (b, c, n) == (4, C, N), (b, c, n)
    xf = np.ascontiguousarray(x.reshape(b, c, n), dtype=np.float32)
    wcat = np.ascontiguousarray(
        np.concatenate(
            [
                np.asarray(w_theta, dtype=np.float32).T,
                np.asarray(w_phi, dtype=np.float32).T,
                np.asarray(w_g, dtype=np.float32).T,
            ],
            axis=1,
        )
    )
    wo_t = np.ascontiguousarray(np.asarray(w_out, dtype=np.float32).T)

    in_maps = []
    for core in range(8):
        bb, q = divmod(core, 2)
        in_maps.append(
            {
                "xf": xf[bb],
                "xq": np.ascontiguousarray(xf[bb][:, q * I : (q + 1) * I]),
                "wcat": wcat,
                "wo_t": wo_t,
            }
        )
    return in_maps


def kernel(x, w_theta, w_phi, w_g, w_out):
    b = x.shape[0]
    c = x.shape[1]
    n = x.shape[2] * x.shape[3]
    in_maps = make_in_maps(x, w_theta, w_phi, w_g, w_out)

    nc = _get_program()
    res = run_bass_kernel_spmd(nc, in_maps, core_ids=list(range(8)))

    out_full = np.empty((b, c, n), dtype=np.float32)
    for core in range(8):
        bb, q = divmod(core, 2)
        out_full[bb][:, q * I : (q + 1) * I] = res.results[core]["out"]
    return out_full.reshape(x.shape)


# revision 33
# speedup vs baseline: 1.0250x; 1.0153x over previous
"""NonLocalBlock (non-local attention) Trainium2 kernel.

Problem: x[4, 256, 64, 64] f32; 1x1-conv projections theta/phi/g (mid=128),
attention over n = h*w = 4096 positions per batch element, output projection
and residual add.

Sharding: data-parallel over batch (4) x query-halves (2) = 8 cores.
Each core gets the full key/value side (xf = x[b] as [256, 4096]) and a
2048-query slice; it computes out[b][:, i_range] = x + w_out @ y.

Per-core algorithm (all layouts chosen to avoid on-chip transposes):
  - theta[m, i] = w_theta @ xq, phi[m, j] = w_phi @ xf  (native fp32 matmuls)
    then split into fp16 hi/lo pairs so the big scores matmul runs as three
    full-rate 16-bit passes (hi*hi + hi*lo + lo*hi); the dropped lo*lo term
    is ~2^-22 relative, i.e. fp32-grade scores.
  - g^T[j, m] computed directly transposed: lhsT = xf block, rhs = w_g^T.
  - scores_T[j, i] = phi^T theta per (j-block 128, i-tile 512); two j-blocks
    share one 2-bank PSUM tile so exp / denominator-accumulate run as one
    [128, 1024] instruction each (halves ACT/DVE per-instruction overhead).
    exp on ACT f32->bf16 with a constant -30 bias instead of a row-max
    subtraction (softmax-invariant; scores reach ~+-80 here, ACT Exp is
    accurate to ~1e-5 over that range, and the shift keeps exp and the
    denominator sums far from f32/bf16 overflow).
  - denominator: DVE accumulates exp tiles into acc (f32), then ones-vector
    matmuls reduce over partitions; reciprocal; K=1 ones matmul broadcasts
    1/denom to all partitions.
  - y[m, i] accumulated over 32 j-blocks in PSUM: lhsT = g^T fp16 block,
    rhs = exp bf16 tile (mixed 16-bit matmul, exact); then scaled by the
    broadcast 1/denom on DVE.
  - out = w_out @ y (fp32) + xq residual, DMA to DRAM.
"""

import numpy as np

import concourse.bacc as bacc
import concourse.bass as bass
import concourse.mybir as mybir
import concourse.tile as tile
from concourse.bass import ts
from concourse.bass_utils import run_bass_kernel_spmd

F32 = mybir.dt.float32
F16 = mybir.dt.float16
BF16 = mybir.dt.bfloat16
AF = mybir.ActivationFunctionType
OP = mybir.AluOpType

P = 128
C = 256  # in channels
MID = 128  # projection channels
N = 4096  # keys per batch element (h*w)
I = 2048  # queries per core
FD = 512  # matmul free-dim tile
FD2 = 1024  # paired tile width
NB_J = N // P  # 32 j-blocks
NT_I = I // FD  # 4 i-tiles


def _build_program() -> bass.Bass:
    # Bacc (not plain Bass): its compile() pass pipeline legalizes semaphore
    # waits (generate_event_semaphores) for this compiler's 1-wait-per-
    # instruction limit.
    nc = bacc.Bacc("TRN2", target_bir_lowering=False)
    xf = nc.dram_tensor("xf", [C, N], F32, kind="ExternalInput")
    xq = nc.dram_tensor("xq", [C, I], F32, kind="ExternalInput")
    # wcat = [w_theta.T | w_phi.T | w_g.T] concatenated on axis 1 so all
    # projection weights arrive in one DMA per 128-channel block.
    wcat = nc.dram_tensor("wcat", [C, 3 * MID], F32, kind="ExternalInput")
    wo = nc.dram_tensor("wo_t", [MID, C], F32, kind="ExternalInput")
    out = nc.dram_tensor("out", [C, I], F32, kind="ExternalOutput")

    with tile.TileContext(nc) as tc:
        with (
            tc.tile_pool(name="singles", bufs=1) as singles,
            tc.tile_pool(name="work", bufs=4) as work,
            tc.tile_pool(name="expp", bufs=16) as expp,
            tc.tile_pool(name="ps", bufs=2, space="PSUM") as ps_pool,
            tc.tile_pool(name="psy", bufs=2, space="PSUM") as psy_pool,
            tc.tile_pool(name="pstail", bufs=2, space="PSUM") as tail_pool,
        ):
            # ---- stage 0: loads -------------------------------------------
            # Loads spread over three DMA queues, ordered so each
            # projection's inputs land just before use:
            #   SP (sync):     wcat[cb=0], xf[cb=0] tiles, wo
            #   Pool (gpsimd): wcat[cb=1], xf[cb=1] tiles
            #   ACT (scalar):  xq tiles, issued from inside the stage-1 loop
            wcat_sb = [
                singles.tile([P, 3 * MID], F32, tag=f"wcat{cb}", name=f"wcat_sb{cb}")
                for cb in range(2)
            ]
            # wcat0 on the ACT queue (small; frees SP to start xf0[0],
            # the first phi matmul's other critical input)
            nc.scalar.dma_start(wcat_sb[0][:], wcat[ts(0, P), :])
            nc.gpsimd.dma_start(wcat_sb[1][:], wcat[ts(1, P), :])
            wth_sb = [wcat_sb[cb][:, 0:MID] for cb in range(2)]
            wph_sb = [wcat_sb[cb][:, MID : 2 * MID] for cb in range(2)]
            wg_sb = [wcat_sb[cb][:, 2 * MID : 3 * MID] for cb in range(2)]

            # xf tiles per channel-block on SP (cb=0) and the gpsimd SWDGE
            # queue (cb=1). xq tiles go on the ACT queue but are issued
            # inside the stage-1 loop (one pair ahead of use) so the issue
            # cost interleaves with ACT compute instead of blocking it.
            xq_sb = [
                singles.tile([P, I], F32, tag=f"xq{cb}", name=f"xq_sb{cb}")
                for cb in range(2)
            ]
            for cb in range(2):
                nc.scalar.dma_start(xq_sb[cb][:, ts(0, FD)], xq[ts(cb, P), ts(0, FD)])
            xf_sb = [
                singles.tile([P, N], F32, tag=f"xf{cb}", name=f"xf_sb{cb}")
                for cb in range(2)
            ]
            engines = [nc.sync, nc.gpsimd]
            for k in range(N // FD):
                for cb in range(2):
                    engines[cb].dma_start(
                        xf_sb[cb][:, ts(k, FD)], xf[ts(cb, P), ts(k, FD)]
                    )
            wo_sb = singles.tile([P, C], F32, tag="wo")
            nc.sync.dma_start(wo_sb[:], wo[:, :])

            ones_col = singles.tile([P, 1], F32, tag="ones_col")
            nc.vector.memset(ones_col[:], 1.0)
            neg30 = singles.tile([P, 1], F32, tag="neg30")
            nc.vector.memset(neg30[:], -30.0)
            ones_row = singles.tile([1, P], F32, tag="ones_row")
            nc.vector.memset(ones_row[:], 1.0)

            # ---- stage 1: projections -------------------------------------
            # Emission interleaved by arrival order of the input tiles so the
            # PE never waits long: per key-tile jt do phi(jt), the fp16 casts,
            # and the four g^T blocks it enables; theta(it=jt) for jt < 4.
            #
            # theta/phi are split into fp16 hi/lo pairs; g^T is computed from
            # fp16 casts of xf / w_g (full-rate matmuls; ~3e-4 elementwise
            # error in g, negligible downstream). The y matmul later takes
            # fp16 stationary x bf16 moving, which is exact on the PE.
            th_hi = singles.tile([P, I], F16, tag="th_hi")
            th_lo = singles.tile([P, I], F16, tag="th_lo")
            ph_hi = singles.tile([P, N], F16, tag="ph_hi")
            ph_lo = singles.tile([P, N], F16, tag="ph_lo")
            gt = singles.tile([P, NB_J, MID], F16, tag="gt")
            xf_f16 = [
                singles.tile([P, N], F16, tag=f"xf16_{cb}", name=f"xf_f16_{cb}")
                for cb in range(2)
            ]
            wg_f16 = [
                singles.tile([P, MID], F16, tag=f"wg16_{cb}", name=f"wg_f16_{cb}")
                for cb in range(2)
            ]
            for cb in range(2):
                nc.scalar.copy(wg_f16[cb][:], wg_sb[cb][:])

            for jt in range(N // FD):
                sl = ts(jt, FD)
                # phi(jt)
                psp = ps_pool.tile([P, FD2], F32, tag="pss")
                nc.tensor.matmul(
                    psp[:, :FD], wph_sb[0][:], xf_sb[0][:, sl],
                    start=True, stop=False,
                )
                nc.tensor.matmul(
                    psp[:, :FD], wph_sb[1][:], xf_sb[1][:, sl],
                    start=False, stop=True,
                )
                nc.scalar.copy(ph_hi[:, sl], psp[:, :FD])
                nc.vector.tensor_tensor(
                    ph_lo[:, sl], psp[:, :FD], ph_hi[:, sl], op=OP.subtract
                )

                # theta(it=jt) for the first four tiles
                if jt < NT_I:
                    if jt + 1 < NT_I:
                        nsl = ts(jt + 1, FD)
                        for cb in range(2):
                            nc.scalar.dma_start(
                                xq_sb[cb][:, nsl], xq[ts(cb, P), nsl]
                            )
                    pst = ps_pool.tile([P, FD2], F32, tag="pss")
                    nc.tensor.matmul(
                        pst[:, :FD], wth_sb[0][:], xq_sb[0][:, sl],
                        start=True, stop=False,
                    )
                    nc.tensor.matmul(
                        pst[:, :FD], wth_sb[1][:], xq_sb[1][:, sl],
                        start=False, stop=True,
                    )
                    nc.scalar.copy(th_hi[:, sl], pst[:, :FD])
                    nc.vector.tensor_tensor(
                        th_lo[:, sl], pst[:, :FD], th_hi[:, sl], op=OP.subtract
                    )

                # fp16 casts of this xf tile, then the 4 g^T blocks it covers
                nc.scalar.copy(xf_f16[0][:, sl], xf_sb[0][:, sl])
                nc.gpsimd.tensor_copy(xf_f16[1][:, sl], xf_sb[1][:, sl])
                for jb in range(4 * jt, 4 * jt + 4):
                    psg = tail_pool.tile([P, FD], F32, tag="tail")
                    nc.tensor.matmul(
                        psg[:, :MID], xf_f16[0][:, ts(jb, P)], wg_f16[0][:],
                        start=True, stop=False,
                    )
                    nc.tensor.matmul(
                        psg[:, :MID], xf_f16[1][:, ts(jb, P)], wg_f16[1][:],
                        start=False, stop=True,
                    )
                    nc.vector.tensor_copy(gt[:, jb, :], psg[:, :MID])

            # ---- stage 2: attention ---------------------------------------
            for it in range(NT_I):
                th_sl = ts(it, FD)
                last_it = it == NT_I - 1
                # For the last i-tile (whose tail nothing overlaps), the final
                # j-pair is processed as two single-block units feeding a
                # separate small accumulator, so the main denominator matmuls
                # issue early and the end-of-kernel critical chain is short.
                n_pairs = NB_J // 2 - 1 if last_it else NB_J // 2
                psy = psy_pool.tile([P, FD], F32, tag="psy")
                acc = work.tile([P, FD2], F32, tag="acc")
                nc.vector.memset(acc[:], 0.0)
                def emit_y(pr, e):
                    # y accumulation: psy[m, i] += gt[jb]^T @ e[jb half]
                    for h in range(2):
                        jb = 2 * pr + h
                        nc.tensor.matmul(
                            psy[:], gt[:, jb, :], e[:, ts(h, FD)],
                            start=(jb == 0), stop=(jb == NB_J - 1),
                        )

                prev = None  # software-pipeline: emit y(pr-1) after the
                # scores of pr so the PE never waits on the just-issued exp
                for pr in range(n_pairs):
                    pss = ps_pool.tile([P, FD2], F32, tag="pss")
                    for h in range(2):
                        jb = 2 * pr + h
                        sl = ts(h, FD)
                        nc.tensor.matmul(
                            pss[:, sl], ph_hi[:, ts(jb, P)], th_hi[:, th_sl],
                            start=True, stop=False,
                        )
                        nc.tensor.matmul(
                            pss[:, sl], ph_hi[:, ts(jb, P)], th_lo[:, th_sl],
                            start=False, stop=False,
                        )
                        nc.tensor.matmul(
                            pss[:, sl], ph_lo[:, ts(jb, P)], th_hi[:, th_sl],
                            start=False, stop=True,
                        )
                    if prev is not None:
                        emit_y(*prev)
                    e = expp.tile([P, FD2], BF16, tag="e")
                    # constant -30 bias: softmax-invariant, keeps exp and the
                    # denominator far from f32/bf16 overflow (scores reach
                    # ~±80 on N(0,1)-scale inputs; without a shift the
                    # denominator sum comes within ~10x of f32 max)
                    nc.scalar.activation(e[:], pss[:], func=AF.Exp, bias=neg30[:])
                    # denominator partials, both halves at once (f32 += bf16)
                    nc.vector.tensor_tensor(acc[:], acc[:], e[:], op=OP.add)
                    prev = (pr, e)

                if prev is not None:
                    emit_y(*prev)

                # denominator: reduce acc over partitions, recip, broadcast
                psd = tail_pool.tile([P, FD], F32, tag="tail")
                if last_it:
                    acc_last = work.tile([P, FD], F32, tag="accl")
                    nc.vector.memset(acc_last[:], 0.0)
                    for jb in range(2 * n_pairs, NB_J):
                        pss1 = ps_pool.tile([P, FD2], F32, tag="pss")
                        nc.tensor.matmul(
                            pss1[:, :FD], ph_hi[:, ts(jb, P)], th_hi[:, th_sl],
                            start=True, stop=False,
                        )
                        nc.tensor.matmul(
                            pss1[:, :FD], ph_hi[:, ts(jb, P)], th_lo[:, th_sl],
                            start=False, stop=False,
                        )
                        nc.tensor.matmul(
                            pss1[:, :FD], ph_lo[:, ts(jb, P)], th_hi[:, th_sl],
                            start=False, stop=True,
                        )
                        if jb == 2 * n_pairs:
                            # main-acc reduction issues here: acc has long
                            # been final, and the PE still has the singles'
                            # score/y matmuls to chew on afterwards
                            nc.tensor.matmul(
                                psd[:1, :], ones_col[:], acc[:, :FD],
                                start=True, stop=False,
                            )
                            nc.tensor.matmul(
                                psd[:1, :], ones_col[:], acc[:, FD:],
                                start=False, stop=False,
                            )
                        e1 = work.tile([P, FD], BF16, tag="e1")
                        nc.scalar.activation(
                            e1[:], pss1[:, :FD], func=AF.Exp, bias=neg30[:]
                        )
                        nc.vector.tensor_tensor(
                            acc_last[:], acc_last[:], e1[:], op=OP.add
                        )
                        nc.tensor.matmul(
                            psy[:], gt[:, jb, :], e1[:],
                            start=False, stop=(jb == NB_J - 1),
                        )
                    nc.tensor.matmul(
                        psd[:1, :], ones_col[:], acc_last[:],
                        start=False, stop=True,
                    )
                else:
                    nc.tensor.matmul(
                        psd[:1, :], ones_col[:], acc[:, :FD],
                        start=True, stop=False,
                    )
                    nc.tensor.matmul(
                        psd[:1, :], ones_col[:], acc[:, FD:],
                        start=False, stop=True,
                    )
                rden = work.tile([1, FD], F32, tag="rden")
                nc.vector.reciprocal(rden[:], psd[:1, :])
                psb = tail_pool.tile([P, FD], F32, tag="tail")
                nc.tensor.matmul(
                    psb[:], ones_row[:], rden[:], start=True, stop=True
                )
                rb_sb = work.tile([P, FD], F32, tag="rb")
                nc.scalar.copy(rb_sb[:], psb[:])

                # normalize y while copying PSUM -> SBUF
                y_sb = work.tile([P, FD], F32, tag="y")
                nc.vector.tensor_tensor(y_sb[:], psy[:], rb_sb[:], op=OP.mult)

                # out = w_out @ y + xq (residual), per 128-channel block
                for cb in range(2):
                    pso = tail_pool.tile([P, FD], F32, tag="tail")
                    nc.tensor.matmul(
                        pso[:], wo_sb[:, ts(cb, P)], y_sb[:],
                        start=True, stop=True,
                    )
                    o_sb = work.tile([P, FD], F32, tag="o")
                    nc.vector.tensor_tensor(
                        o_sb[:], pso[:], xq_sb[cb][:, th_sl], op=OP.add
                    )
                    nc.sync.dma_start(
                        out[ts(cb, P), it * FD : it * FD + 256], o_sb[:, :256]
                    )
                    nc.gpsimd.dma_start(
                        out[ts(cb, P), it * FD + 256 : (it + 1) * FD],
                        o_sb[:, 256:],
                    )

    nc.compile()
    return nc


_CACHED_NC = None


def _get_program():
    global _CACHED_NC
    if _CACHED_NC is None:
        _CACHED_NC = _build_program()
    return _CACHED_NC


def make_in_maps(x, w_theta, w_phi, w_g, w_out):
    b, c, h, w = x.shape
    n = h * w
    assert (b, c, n) == (4, C, N), (b, c, n)
    xf = np.ascontiguousarray(x.reshape(b, c, n), dtype=np.float32)
    wcat = np.ascontiguousarray(
        np.concatenate(
            [
                np.asarray(w_theta, dtype=np.float32).T,
                np.asarray(w_phi, dtype=np.float32).T,
                np.asarray(w_g, dtype=np.float32).T,
            ],
            axis=1,
        )
    )
    wo_t = np.ascontiguousarray(np.asarray(w_out, dtype=np.float32).T)

    in_maps = []
    for core in range(8):
        bb, q = divmod(core, 2)
        in_maps.append(
            {
                "xf": xf[bb],
                "xq": np.ascontiguousarray(xf[bb][:, q * I : (q + 1) * I]),
                "wcat": wcat,
                "wo_t": wo_t,
            }
        )
    return in_maps


def kernel(x, w_theta, w_phi, w_g, w_out):
    b = x.shape[0]
    c = x.shape[1]
    n = x.shape[2] * x.shape[3]
    in_maps = make_in_maps(x, w_theta, w_phi, w_g, w_out)

    nc = _get_program()
    res = run_bass_kernel_spmd(nc, in_maps, core_ids=list(range(8)))

    out_full = np.empty((b, c, n), dtype=np.float32)
    for core in range(8):
        bb, q = divmod(core, 2)
        out_full[bb][:, q * I : (q + 1) * I] = res.results[core]["out"]
    return out_full.reshape(x.shape)


# revision 40
# speedup vs baseline: 1.0528x; 1.0271x over previous
"""NonLocalBlock (non-local attention) Trainium2 kernel.

Problem: x[4, 256, 64, 64] f32; 1x1-conv projections theta/phi/g (mid=128),
attention over n = h*w = 4096 positions per batch element, output projection
and residual add.

Sharding: data-parallel over batch (4) x query-halves (2) = 8 cores.
Each core gets the full key/value side (xf = x[b] as [256, 4096]) and a
2048-query slice; it computes out[b][:, i_range] = x + w_out @ y.

Per-core algorithm (all layouts chosen to avoid on-chip transposes):
  - theta[m, i] = w_theta @ xq, phi[m, j] = w_phi @ xf  (native fp32 matmuls)
    then split into fp16 hi/lo pairs so the big scores matmul runs as three
    full-rate 16-bit passes (hi*hi + hi*lo + lo*hi); the dropped lo*lo term
    is ~2^-22 relative, i.e. fp32-grade scores.
  - g^T[j, m] computed directly transposed: lhsT = xf block, rhs = w_g^T.
  - scores_T[j, i] = phi^T theta per (j-block 128, i-tile 512); two j-blocks
    share one 2-bank PSUM tile so exp / denominator-accumulate run as one
    [128, 1024] instruction each (halves ACT/DVE per-instruction overhead).
    exp on ACT f32->bf16 with a constant -30 bias instead of a row-max
    subtraction (softmax-invariant; scores reach ~+-80 here, ACT Exp is
    accurate to ~1e-5 over that range, and the shift keeps exp and the
    denominator sums far from f32/bf16 overflow).
  - denominator: DVE accumulates exp tiles into acc (f32), then ones-vector
    matmuls reduce over partitions; reciprocal; K=1 ones matmul broadcasts
    1/denom to all partitions.
  - y[m, i] accumulated over 32 j-blocks in PSUM: lhsT = g^T fp16 block,
    rhs = exp bf16 tile (mixed 16-bit matmul, exact); then scaled by the
    broadcast 1/denom on DVE.
  - out = w_out @ y (fp32) + xq residual, DMA to DRAM.
"""

import numpy as np

import concourse.bacc as bacc
import concourse.bass as bass
import concourse.mybir as mybir
import concourse.tile as tile
from concourse.bass import ts
from concourse.bass_utils import run_bass_kernel_spmd

F32 = mybir.dt.float32
F16 = mybir.dt.float16
BF16 = mybir.dt.bfloat16
AF = mybir.ActivationFunctionType
OP = mybir.AluOpType

P = 128
C = 256  # in channels
MID = 128  # projection channels
N = 4096  # keys per batch element (h*w)
I = 2048  # queries per core
FD = 512  # matmul free-dim tile
FD2 = 1024  # paired tile width
NB_J = N // P  # 32 j-blocks
NT_I = I // FD  # 4 i-tiles


def _build_program() -> bass.Bass:
    # Bacc (not plain Bass): its compile() pass pipeline legalizes semaphore
    # waits (generate_event_semaphores) for this compiler's 1-wait-per-
    # instruction limit.
    nc = bacc.Bacc("TRN2", target_bir_lowering=False)
    xf = nc.dram_tensor("xf", [C, N], F32, kind="ExternalInput")
    xq = nc.dram_tensor("xq", [C, I], F32, kind="ExternalInput")
    # wcat = [w_theta.T | w_phi.T | w_g.T] concatenated on axis 1 so all
    # projection weights arrive in one DMA per 128-channel block.
    wcat = nc.dram_tensor("wcat", [C, 3 * MID], F32, kind="ExternalInput")
    wo = nc.dram_tensor("wo_t", [MID, C], F32, kind="ExternalInput")
    out = nc.dram_tensor("out", [C, I], F32, kind="ExternalOutput")

    with tile.TileContext(nc) as tc:
        with (
            tc.tile_pool(name="singles", bufs=1) as singles,
            tc.tile_pool(name="work", bufs=4) as work,
            tc.tile_pool(name="expp", bufs=16) as expp,
            tc.tile_pool(name="ps", bufs=2, space="PSUM") as ps_pool,
            tc.tile_pool(name="psy", bufs=2, space="PSUM") as psy_pool,
            tc.tile_pool(name="pstail", bufs=2, space="PSUM") as tail_pool,
        ):
            # ---- stage 0: loads -------------------------------------------
            # Loads spread over three DMA queues, ordered so each
            # projection's inputs land just before use:
            #   SP (sync):     wcat[cb=0], xf[cb=0] tiles, wo
            #   Pool (gpsimd): wcat[cb=1], xf[cb=1] tiles
            #   ACT (scalar):  xq tiles, issued from inside the stage-1 loop
            wcat_sb = [
                singles.tile([P, 3 * MID], F32, tag=f"wcat{cb}", name=f"wcat_sb{cb}")
                for cb in range(2)
            ]
            # wcat0 on the ACT queue (small; frees SP to start xf0[0],
            # the first phi matmul's other critical input)
            nc.scalar.dma_start(wcat_sb[0][:], wcat[ts(0, P), :])
            nc.gpsimd.dma_start(wcat_sb[1][:], wcat[ts(1, P), :])
            wth_sb = [wcat_sb[cb][:, 0:MID] for cb in range(2)]
            wph_sb = [wcat_sb[cb][:, MID : 2 * MID] for cb in range(2)]
            wg_sb = [wcat_sb[cb][:, 2 * MID : 3 * MID] for cb in range(2)]

            # xf tiles per channel-block on SP (cb=0) and the gpsimd SWDGE
            # queue (cb=1). xq tiles go on the ACT queue but are issued
            # inside the stage-1 loop (one pair ahead of use) so the issue
            # cost interleaves with ACT compute instead of blocking it.
            xq_sb = [
                singles.tile([P, I], F32, tag=f"xq{cb}", name=f"xq_sb{cb}")
                for cb in range(2)
            ]
            for cb in range(2):
                nc.scalar.dma_start(xq_sb[cb][:, ts(0, FD)], xq[ts(cb, P), ts(0, FD)])
            xf_sb = [
                singles.tile([P, N], F32, tag=f"xf{cb}", name=f"xf_sb{cb}")
                for cb in range(2)
            ]
            engines = [nc.sync, nc.gpsimd]
            for k in range(N // FD):
                for cb in range(2):
                    engines[cb].dma_start(
                        xf_sb[cb][:, ts(k, FD)], xf[ts(cb, P), ts(k, FD)]
                    )
            wo_sb = singles.tile([P, C], F32, tag="wo")
            nc.sync.dma_start(wo_sb[:], wo[:, :])

            ones_col = singles.tile([P, 1], F32, tag="ones_col")
            nc.vector.memset(ones_col[:], 1.0)
            neg30 = singles.tile([P, 1], F32, tag="neg30")
            nc.vector.memset(neg30[:], -30.0)
            ones_row = singles.tile([1, P], F32, tag="ones_row")
            nc.vector.memset(ones_row[:], 1.0)
            wo_f16 = singles.tile([P, C], F16, tag="wo16")

            # ---- stage 1: projections -------------------------------------
            # Emission interleaved by arrival order of the input tiles so the
            # PE never waits long: per key-tile jt do phi(jt), the fp16 casts,
            # and the four g^T blocks it enables; theta(it=jt) for jt < 4.
            #
            # theta/phi are split into fp16 hi/lo pairs; g^T is computed from
            # fp16 casts of xf / w_g (full-rate matmuls; ~3e-4 elementwise
            # error in g, negligible downstream). The y matmul later takes
            # fp16 stationary x bf16 moving, which is exact on the PE.
            th_hi = singles.tile([P, I], F16, tag="th_hi")
            th_lo = singles.tile([P, I], F16, tag="th_lo")
            ph_hi = singles.tile([P, N], F16, tag="ph_hi")
            ph_lo = singles.tile([P, N], F16, tag="ph_lo")
            gt = singles.tile([P, NB_J, MID], F16, tag="gt")
            xf_f16 = [
                singles.tile([P, N], F16, tag=f"xf16_{cb}", name=f"xf_f16_{cb}")
                for cb in range(2)
            ]
            wg_f16 = [
                singles.tile([P, MID], F16, tag=f"wg16_{cb}", name=f"wg_f16_{cb}")
                for cb in range(2)
            ]
            for cb in range(2):
                nc.scalar.copy(wg_f16[cb][:], wg_sb[cb][:])

            for jt in range(N // FD):
                sl = ts(jt, FD)
                # phi(jt)
                psp = ps_pool.tile([P, FD2], F32, tag="pss")
                nc.tensor.matmul(
                    psp[:, :FD], wph_sb[0][:], xf_sb[0][:, sl],
                    start=True, stop=False,
                )
                nc.tensor.matmul(
                    psp[:, :FD], wph_sb[1][:], xf_sb[1][:, sl],
                    start=False, stop=True,
                )
                nc.scalar.copy(ph_hi[:, sl], psp[:, :FD])
                nc.vector.tensor_tensor(
                    ph_lo[:, sl], psp[:, :FD], ph_hi[:, sl], op=OP.subtract
                )

                # theta(it=jt) for the first four tiles
                if jt < NT_I:
                    if jt + 1 < NT_I:
                        nsl = ts(jt + 1, FD)
                        for cb in range(2):
                            nc.scalar.dma_start(
                                xq_sb[cb][:, nsl], xq[ts(cb, P), nsl]
                            )
                    pst = ps_pool.tile([P, FD2], F32, tag="pss")
                    nc.tensor.matmul(
                        pst[:, :FD], wth_sb[0][:], xq_sb[0][:, sl],
                        start=True, stop=False,
                    )
                    nc.tensor.matmul(
                        pst[:, :FD], wth_sb[1][:], xq_sb[1][:, sl],
                        start=False, stop=True,
                    )
                    nc.scalar.copy(th_hi[:, sl], pst[:, :FD])
                    nc.vector.tensor_tensor(
                        th_lo[:, sl], pst[:, :FD], th_hi[:, sl], op=OP.subtract
                    )

                # fp16 casts of this xf tile, then the 4 g^T blocks it covers
                nc.scalar.copy(xf_f16[0][:, sl], xf_sb[0][:, sl])
                nc.gpsimd.tensor_copy(xf_f16[1][:, sl], xf_sb[1][:, sl])
                for jb in range(4 * jt, 4 * jt + 4):
                    psg = tail_pool.tile([P, FD], F32, tag="tail")
                    nc.tensor.matmul(
                        psg[:, :MID], xf_f16[0][:, ts(jb, P)], wg_f16[0][:],
                        start=True, stop=False,
                    )
                    nc.tensor.matmul(
                        psg[:, :MID], xf_f16[1][:, ts(jb, P)], wg_f16[1][:],
                        start=False, stop=True,
                    )
                    nc.vector.tensor_copy(gt[:, jb, :], psg[:, :MID])

            # wo cast emitted here: its DMA lands mid-stage-1, and casting
            # earlier would block ACT's in-order stream (stalling the
            # theta/phi hi copies the PE needs)
            nc.scalar.copy(wo_f16[:], wo_sb[:])

            # ---- stage 2: attention ---------------------------------------
            for it in range(NT_I):
                th_sl = ts(it, FD)
                last_it = it == NT_I - 1
                # For the last i-tile (whose tail nothing overlaps), the final
                # j-pair is processed as two single-block units feeding a
                # separate small accumulator, so the main denominator matmuls
                # issue early and the end-of-kernel critical chain is short.
                n_pairs = NB_J // 2 - 1 if last_it else NB_J // 2
                psy = psy_pool.tile([P, FD], F32, tag="psy")
                acc = work.tile([P, FD2], F32, tag="acc")
                nc.vector.memset(acc[:], 0.0)
                def emit_y(pr, e):
                    # y accumulation: psy[m, i] += gt[jb]^T @ e[jb half]
                    for h in range(2):
                        jb = 2 * pr + h
                        nc.tensor.matmul(
                            psy[:], gt[:, jb, :], e[:, ts(h, FD)],
                            start=(jb == 0), stop=(jb == NB_J - 1),
                        )

                prev = None  # software-pipeline: emit y(pr-1) after the
                # scores of pr so the PE never waits on the just-issued exp
                for pr in range(n_pairs):
                    pss = ps_pool.tile([P, FD2], F32, tag="pss")
                    for h in range(2):
                        jb = 2 * pr + h
                        sl = ts(h, FD)
                        nc.tensor.matmul(
                            pss[:, sl], ph_hi[:, ts(jb, P)], th_hi[:, th_sl],
                            start=True, stop=False,
                        )
                        nc.tensor.matmul(
                            pss[:, sl], ph_hi[:, ts(jb, P)], th_lo[:, th_sl],
                            start=False, stop=False,
                        )
                        nc.tensor.matmul(
                            pss[:, sl], ph_lo[:, ts(jb, P)], th_hi[:, th_sl],
                            start=False, stop=True,
                        )
                    if prev is not None:
                        emit_y(*prev)
                    e = expp.tile([P, FD2], BF16, tag="e")
                    # constant -30 bias: softmax-invariant, keeps exp and the
                    # denominator far from f32/bf16 overflow (scores reach
                    # ~±80 on N(0,1)-scale inputs; without a shift the
                    # denominator sum comes within ~10x of f32 max)
                    nc.scalar.activation(e[:], pss[:], func=AF.Exp, bias=neg30[:])
                    # denominator partials, both halves at once (f32 += bf16)
                    nc.vector.tensor_tensor(acc[:], acc[:], e[:], op=OP.add)
                    prev = (pr, e)

                if prev is not None:
                    emit_y(*prev)

                # denominator: reduce acc over partitions, recip, broadcast
                psd = tail_pool.tile([P, FD], F32, tag="tail")
                if last_it:
                    acc_last = work.tile([P, FD], F32, tag="accl")
                    nc.vector.memset(acc_last[:], 0.0)
                    for jb in range(2 * n_pairs, NB_J):
                        pss1 = ps_pool.tile([P, FD2], F32, tag="pss")
                        nc.tensor.matmul(
                            pss1[:, :FD], ph_hi[:, ts(jb, P)], th_hi[:, th_sl],
                            start=True, stop=False,
                        )
                        nc.tensor.matmul(
                            pss1[:, :FD], ph_hi[:, ts(jb, P)], th_lo[:, th_sl],
                            start=False, stop=False,
                        )
                        nc.tensor.matmul(
                            pss1[:, :FD], ph_lo[:, ts(jb, P)], th_hi[:, th_sl],
                            start=False, stop=True,
                        )
                        if jb == 2 * n_pairs:
                            # main-acc reduction issues here: acc has long
                            # been final, and the PE still has the singles'
                            # score/y matmuls to chew on afterwards
                            nc.tensor.matmul(
                                psd[:1, :], ones_col[:], acc[:, :FD],
                                start=True, stop=False,
                            )
                            nc.tensor.matmul(
                                psd[:1, :], ones_col[:], acc[:, FD:],
                                start=False, stop=False,
                            )
                        e1 = work.tile([P, FD], BF16, tag="e1")
                        nc.scalar.activation(
                            e1[:], pss1[:, :FD], func=AF.Exp, bias=neg30[:]
                        )
                        nc.vector.tensor_tensor(
                            acc_last[:], acc_last[:], e1[:], op=OP.add
                        )
                        nc.tensor.matmul(
                            psy[:], gt[:, jb, :], e1[:],
                            start=False, stop=(jb == NB_J - 1),
                        )
                    nc.tensor.matmul(
                        psd[:1, :], ones_col[:], acc_last[:],
                        start=False, stop=True,
                    )
                else:
                    nc.tensor.matmul(
                        psd[:1, :], ones_col[:], acc[:, :FD],
                        start=True, stop=False,
                    )
                    nc.tensor.matmul(
                        psd[:1, :], ones_col[:], acc[:, FD:],
                        start=False, stop=True,
                    )
                # fp16 broadcast + fp16 output projection: one fp16 rounding
                # of 1/denom and of y/w_out (~2.4e-4 elementwise, no exponent
                # amplification) buys 4x-faster tail matmuls
                rden = work.tile([1, FD], F32, tag="rden")
                nc.vector.reciprocal(rden[:], psd[:1, :])
                psb = tail_pool.tile([P, FD], F32, tag="tail")
                nc.tensor.matmul(
                    psb[:], ones_row[:], rden[:], start=True, stop=True
                )
                rb_sb = work.tile([P, FD], F32, tag="rb")
                nc.scalar.copy(rb_sb[:], psb[:])

                # normalize y while copying PSUM -> SBUF, rounding to fp16
                y_sb = work.tile([P, FD], F16, tag="y")
                nc.vector.tensor_tensor(y_sb[:], psy[:], rb_sb[:], op=OP.mult)

                # out = w_out @ y + xq (residual), per 128-channel block
                for cb in range(2):
                    pso = tail_pool.tile([P, FD], F32, tag="tail")
                    nc.tensor.matmul(
                        pso[:], wo_f16[:, ts(cb, P)], y_sb[:],
                        start=True, stop=True,
                    )
                    o_sb = work.tile([P, FD], F32, tag="o")
                    nc.vector.tensor_tensor(
                        o_sb[:], pso[:], xq_sb[cb][:, th_sl], op=OP.add
                    )
                    nc.sync.dma_start(
                        out[ts(cb, P), it * FD : it * FD + 256], o_sb[:, :256]
                    )
                    nc.gpsimd.dma_start(
                        out[ts(cb, P), it * FD + 256 : (it + 1) * FD],
                        o_sb[:, 256:],
                    )

    nc.compile()
    return nc


_CACHED_NC = None


def _get_program():
    global _CACHED_NC
    if _CACHED_NC is None:
        _CACHED_NC = _build_program()
    return _CACHED_NC


def make_in_maps(x, w_theta, w_phi, w_g, w_out):
    b, c, h, w = x.shape
    n = h * w
    assert (b, c, n) == (4, C, N), (b, c, n)
    xf = np.ascontiguousarray(x.reshape(b, c, n), dtype=np.float32)
    wcat = np.ascontiguousarray(
        np.concatenate(
            [
                np.asarray(w_theta, dtype=np.float32).T,
                np.asarray(w_phi, dtype=np.float32).T,
                np.asarray(w_g, dtype=np.float32).T,
            ],
            axis=1,
        )
    )
    wo_t = np.ascontiguousarray(np.asarray(w_out, dtype=np.float32).T)

    in_maps = []
    for core in range(8):
        bb, q = divmod(core, 2)
        in_maps.append(
            {
                "xf": xf[bb],
                "xq": np.ascontiguousarray(xf[bb][:, q * I : (q + 1) * I]),
                "wcat": wcat,
                "wo_t": wo_t,
            }
        )
    return in_maps


def kernel(x, w_theta, w_phi, w_g, w_out):
    b = x.shape[0]
    c = x.shape[1]
    n = x.shape[2] * x.shape[3]
    in_maps = make_in_maps(x, w_theta, w_phi, w_g, w_out)

    nc = _get_program()
    res = run_bass_kernel_spmd(nc, in_maps, core_ids=list(range(8)))

    out_full = np.empty((b, c, n), dtype=np.float32)
    for core in range(8):
        bb, q = divmod(core, 2)
        out_full[bb][:, q * I : (q + 1) * I] = res.results[core]["out"]
    return out_full.reshape(x.shape)
